# revision 24
# baseline (speedup 1.0000x reference)
"""Causal multi-head attention (B=4, S=2048, D=2048, H=16, RoPE) on 8 TRN2 NeuronCores.

Sharding: core c handles (batch b = c//2, head-group g = c%2) -- 8 heads per core.
Each core computes its head-group's Q/K/V projections (column-sharded weights),
RoPE, causal softmax attention, and the row-sharded Wo partial product.
The host sums the two partial outputs per batch (the "all-reduce") and
transposes back.

Device-side layout is fully transposed ("feature-major"): activations are kept
as [feature, seq] so every GEMM contracts over the partition dimension without
any on-device transposes. All matmul inputs are bf16 (fp32 accumulate in PSUM);
softmax runs in fp32.

Schedule (single core):
  phase 1a: K projections in 4 head-pair passes, d-outer over 8 live PSUM
            banks so the first pass overlaps the X^T DMA stream.
  phase 1b: V projection.
  phase 2:  per-head software pipeline [Q-proj(h) || attention(h-1)].
  phase 3:  attention(7) interleaved with the Wo eo-loop (Wo accumulates
            heads 0..6 while head 7's softmax drains).
RoPE epilogues drain PSUM via the Vector engine (cos-mul) + two Scalar
half-swap copies, keeping the Scalar engine free for the attention exp chain.
"""

import math
import sys
import types

import numpy as np
import ml_dtypes

BF16 = ml_dtypes.bfloat16

S = 2048
D = 2048
H = 16
DK = 128
B = 4
E = 1024          # head-group width (8 heads x 128)
HPC = 8           # heads per core
NT_D = 16         # 128-wide tiles along the contraction (model) dim
NT_S4 = 4         # 512-wide tiles along seq
NT_S16 = 16       # 128-wide tiles along seq
ROPE_THETA = 10000.0

# Set by test harness to capture a profile; kernel() then stores results here.
TRACE = False
LAST_RESULT = None

_PROGRAM_CACHE = {}


def _install_ntff_hook():
    """Register the NTFF profile hook that this image's antenv lacks."""
    if "antenv.axon_hooks" in sys.modules:
        return
    holder = {"hook": None}
    mod = types.ModuleType("antenv.axon_hooks")
    mod.set_axon_ntff_profile_hook = lambda h: holder.__setitem__("hook", h)
    mod.get_axon_ntff_profile_hook = lambda: holder["hook"]
    sys.modules["antenv.axon_hooks"] = mod
    try:
        from trn_agent_boot.trn_boot import _ntff_profile_via_ctypes

        mod.set_axon_ntff_profile_hook(
            _ntff_profile_via_ctypes("/opt/axon/libaxon_pjrt.so")
        )
    except Exception:
        pass


def _build_program():
    """Build + compile the single-core Bass program (same program on all 8 cores)."""
    if "nc" in _PROGRAM_CACHE:
        return _PROGRAM_CACHE["nc"]

    from contextlib import ExitStack

    import concourse.mybir as mybir
    import concourse.tile as tile
    from concourse import bacc

    F32 = mybir.dt.float32
    B16 = mybir.dt.bfloat16

    nc = bacc.Bacc("TRN2", target_bir_lowering=False, debug=False, num_devices=8)

    # Weights are pre-transposed on the host into partition-major layout so
    # every weight DMA moves contiguous 4KB partition lines at full bandwidth
    # (the blocked [d, p, f] layout produced 256B scatter packets at ~80GB/s).
    xt = nc.dram_tensor("xt", [NT_D, 128, S], B16, kind="ExternalInput").ap()
    wq = nc.dram_tensor("wq", [HPC, 128, NT_D * 128], B16, kind="ExternalInput").ap()
    wk = nc.dram_tensor("wk", [HPC, 128, NT_D * 128], B16, kind="ExternalInput").ap()
    wv = nc.dram_tensor("wv", [NT_D, 128, E], B16, kind="ExternalInput").ap()
    wo = nc.dram_tensor("wo", [NT_D, 128, HPC * 128], B16, kind="ExternalInput").ap()
    cos = nc.dram_tensor("cos", [128, S], B16, kind="ExternalInput").ap()
    sin = nc.dram_tensor("sin", [128, S], B16, kind="ExternalInput").ap()
    msk = nc.dram_tensor("msk", [128, 512], B16, kind="ExternalInput").ap()
    ones = nc.dram_tensor("ones", [128, 128], B16, kind="ExternalInput").ap()
    outt = nc.dram_tensor("outt", [D, S], F32, kind="ExternalOutput").ap()

    Exp = mybir.ActivationFunctionType.Exp

    with tile.TileContext(nc, pool_alloc_mode="queue") as tc, ExitStack() as ctx:
        cpool = ctx.enter_context(tc.tile_pool(name="const", bufs=1))

        # Persistent activation stores (bf16).
        kt_pool = ctx.enter_context(tc.tile_pool(name="kt", bufs=HPC))
        v_pool = ctx.enter_context(tc.tile_pool(name="v", bufs=NT_S16))
        kts, vts, ats = [], [], []

        def load_w(wdram, e, nm, w_pool):
            wt = w_pool.tile([128, NT_D * 128], B16, tag="w", name=f"w{nm}_{e}")
            nc.sync.dma_start(out=wt, in_=wdram[e])
            return wt

        def rope_epilogue(ps, qh, s4, nm, e, r_pool):
            """RoPE: qh[:, sl] = ps*cos + swap_halves(ps)*sin.

            DVE drains the PSUM directly for the cos term; ACT only does the
            two half-swap copies, so the exp chain isn't starved at head
            boundaries.
            """
            sl = slice(s4 * 512, (s4 + 1) * 512)
            t1 = r_pool.tile([128, 512], B16, tag="t2", name=f"t1_{nm}_{e}_{s4}")
            qs = r_pool.tile([128, 512], B16, tag="u", name=f"qs_{nm}_{e}_{s4}")
            u = r_pool.tile([128, 512], B16, tag="u2", name=f"u_{nm}_{e}_{s4}")
            nc.vector.tensor_mul(out=t1, in0=ps, in1=cos_t[:, sl])
            nc.scalar.copy(out=qs[0:64, :], in_=ps[64:128, :])
            nc.scalar.copy(out=qs[64:128, :], in_=ps[0:64, :])
            nc.vector.tensor_mul(out=u, in0=qs, in1=sin_t[:, sl])
            nc.vector.tensor_add(out=qh[:, sl], in0=t1, in1=u)

        def attention(h, qh, at, pt_pool, rc_pool, ps_pool, pbufs, s4_range,
                      fillers=None):
            """Causal attention for head h into at ([dv=128, S] bf16).

            pbufs: dict of bufs per PSUM tag (psc/pat/pde). fillers: optional
            list of thunks; one is emitted after each sk tile so independent
            matmuls sit between the exp-gated ones in the engine queues.
            """
            for s4 in s4_range:
                nsk = 4 * s4 + 4
                sl = slice(s4 * 512, (s4 + 1) * 512)
                pat = ps_pool.tile([128, 512], F32, tag="pat", bufs=pbufs["pat"],
                                   name=f"pat_{h}_{s4}")
                pde = ps_pool.tile([128, 512], F32, tag="pde", bufs=pbufs["pde"],
                                   name=f"pde_{h}_{s4}")
                for sk in range(nsk):
                    # Diagonal tiles: columns j < 128*r are fully masked; skip
                    # them in all three matmuls.
                    r = sk - 4 * s4
                    off = 128 * r if r > 0 else 0
                    w = 512 - off
                    psc_t = ps_pool.tile([128, 512], F32, tag="psc",
                                         bufs=pbufs["psc"],
                                         name=f"psc_{h}_{s4}_{sk}")
                    psc = psc_t[:, 0:w]
                    nc.tensor.matmul(
                        psc,
                        lhsT=kts[h][:, sk * 128 : (sk + 1) * 128],
                        rhs=qh[:, s4 * 512 + off : (s4 + 1) * 512],
                        start=True, stop=True,
                    )
                    pt_t = pt_pool.tile([128, 512], B16, tag="pt",
                                        name=f"pt_{h}_{s4}_{sk}")
                    pt = pt_t[:, 0:w]
                    # Promote the exp+mask chain: the attnV matmul needs the
                    # masked tile now.
                    with tc.high_priority(offset=400):
                        nc.scalar.activation(out=pt, in_=psc, func=Exp)
                        if r >= 0:
                            # Only the first 128 columns of the surviving
                            # diagonal strip need the triangle mask.
                            mw = min(128, w)
                            nc.vector.tensor_mul(
                                out=pt[:, 0:mw], in0=pt[:, 0:mw],
                                in1=msk_t[:, 0:mw],
                            )
                    nc.tensor.matmul(
                        pat[:, off:512],
                        lhsT=vts[sk][:, h * 128 : (h + 1) * 128],
                        rhs=pt, start=(sk == 0), stop=(sk == nsk - 1),
                    )
                    nc.tensor.matmul(
                        pde[:, off:512], lhsT=one_t, rhs=pt,
                        start=(sk == 0), stop=(sk == nsk - 1),
                    )
                    if fillers:
                        fillers.pop(0)()
                rcb = rc_pool.tile([128, 512], F32, tag="rcb",
                                   name=f"rcb_{h}_{s4}")
                nc.vector.reciprocal_approx_fast(out=rcb, in_=pde)
                nc.vector.tensor_mul(out=at[:, sl], in0=pat, in1=rcb)
            while fillers:
                fillers.pop(0)()

        # ---------------- Phase 1a: load X^T, K projections ----------------
        with ExitStack() as p1ctx:
            xt_pool = p1ctx.enter_context(tc.tile_pool(name="xt", bufs=NT_D))
            w_pool = p1ctx.enter_context(tc.tile_pool(name="wst", bufs=3))
            r_pool = p1ctx.enter_context(tc.tile_pool(name="rope", bufs=2))
            # DMA priming order: K weights for head-pair 0 first, then X^T
            # tiles, then the small constants (needed only once RoPE starts).
            wt0 = w_pool.tile([128, NT_D * 128], B16, tag="w", name="wk_0")
            for c in range(4):
                nc.sync.dma_start(
                    out=wt0[:, c * 512 : (c + 1) * 512],
                    in_=wk[0][:, c * 512 : (c + 1) * 512],
                )
            wt1 = load_w(wk, 1, "k", w_pool)
            xts = []
            for d in range(NT_D):
                xtile = xt_pool.tile([128, S], B16, tag="xt", name=f"xt_{d}")
                if d == 0:
                    for c in range(4):
                        nc.sync.dma_start(
                            out=xtile[:, c * 512 : (c + 1) * 512],
                            in_=xt[0][:, c * 512 : (c + 1) * 512],
                        )
                else:
                    nc.sync.dma_start(out=xtile, in_=xt[d])
                xts.append(xtile)

            cos_t = cpool.tile([128, S], B16, tag="cos", name="cos_t")
            nc.sync.dma_start(out=cos_t, in_=cos)
            sin_t = cpool.tile([128, S], B16, tag="sin", name="sin_t")
            nc.sync.dma_start(out=sin_t, in_=sin)
            msk_t = cpool.tile([128, 512], B16, tag="msk", name="msk_t")
            nc.sync.dma_start(out=msk_t, in_=msk)
            one_t = cpool.tile([128, 128], B16, tag="one", name="one_t")
            nc.sync.dma_start(out=one_t, in_=ones)

            # K projections. Pass 0 (heads 0+1) runs d-outer over 8 live PSUM
            # banks so compute consumes each xt[d] as its DMA lands; heads
            # 2..7 use the column-pair structure so the RoPE drains pipeline
            # against the next head's matmuls instead of bursting. One shared
            # pool (single tag) for all of phase 1 — no release barrier.
            kv_ps = tc.alloc_tile_pool(name="kvps", bufs=8, space="PSUM")

            def kv_tile(nm):
                return kv_ps.tile([128, 512], F32, tag="p8", bufs=8, name=nm)

            psums = [kv_tile(f"pk0_{i}") for i in range(8)]
            for d in range(NT_D):
                for ei, wt in ((0, wt0), (1, wt1)):
                    lhsT = wt[:, d * 128 : (d + 1) * 128]
                    for s4 in range(4):
                        nc.tensor.matmul(
                            psums[4 * ei + s4], lhsT=lhsT,
                            rhs=xts[d][:, s4 * 512 : (s4 + 1) * 512],
                            start=(d == 0), stop=(d == NT_D - 1),
                        )
            for ei, e in ((0, 0), (1, 1)):
                kh = kt_pool.tile([128, S], B16, tag="kt", name=f"kh_{e}")
                for s4 in range(4):
                    rope_epilogue(psums[4 * ei + s4], kh, s4, "k", e, r_pool)
                kts.append(kh)

            for e in range(2, HPC):
                wt = load_w(wk, e, "k", w_pool)
                kh = kt_pool.tile([128, S], B16, tag="kt", name=f"kh_{e}")
                for sp in range(2):
                    ps2 = [kv_tile(f"pk_{e}_{sp}_{s2}") for s2 in range(2)]
                    for d in range(NT_D):
                        lhsT = wt[:, d * 128 : (d + 1) * 128]
                        for s2 in range(2):
                            s4 = 2 * sp + s2
                            nc.tensor.matmul(
                                ps2[s2], lhsT=lhsT,
                                rhs=xts[d][:, s4 * 512 : (s4 + 1) * 512],
                                start=(d == 0), stop=(d == NT_D - 1),
                            )
                    for s2 in range(2):
                        rope_epilogue(ps2[s2], kh, 2 * sp + s2, "k", e, r_pool)
                kts.append(kh)

            # ------------- Phase 1b: V projection -------------
            with tc.tile_pool(name="wvp", bufs=NT_D) as wv_pool:
                wvts = []
                for d in range(NT_D):
                    wvt = wv_pool.tile([128, E], B16, tag="wv", name=f"wv_{d}")
                    nc.sync.dma_start(out=wvt, in_=wv[d])
                    wvts.append(wvt)
                for s in range(NT_S16):
                    pv = [kv_tile(f"pv_{s}_{i}") for i in range(2)]
                    for d in range(NT_D):
                        lhsT = xts[d][:, s * 128 : (s + 1) * 128]
                        for i in range(2):
                            nc.tensor.matmul(
                                pv[i], lhsT=lhsT,
                                rhs=wvts[d][:, i * 512 : (i + 1) * 512],
                                start=(d == 0), stop=(d == NT_D - 1),
                            )
                    vt = v_pool.tile([128, E], B16, tag="vt", name=f"vt_{s}")
                    nc.scalar.copy(out=vt[:, 0:512], in_=pv[0])
                    nc.scalar.copy(out=vt[:, 512:1024], in_=pv[1])
                    vts.append(vt)
            kv_ps.release()

            # ------- Phase 2: per-head pipeline: Q-proj(h) + attn(h-1) -------
            # These pools open here (after the transient wv pool freed its
            # ring space) but live on the outer ctx: phase 3 still needs
            # qh7/at*/pt/rc.
            attn_pool = ctx.enter_context(
                tc.tile_pool(name="attn", bufs=HPC, side="right")
            )
            qt_pool = ctx.enter_context(
                tc.tile_pool(name="qt", bufs=2, side="right")
            )
            pt_pool = ctx.enter_context(
                tc.tile_pool(name="pt", bufs=5, side="right")
            )
            rc_pool = ctx.enter_context(
                tc.tile_pool(name="rcp", bufs=2, side="right")
            )
            ps_pool = p1ctx.enter_context(
                tc.tile_pool(name="ps", bufs=2, space="PSUM")
            )
            P2B = {"psc": 2, "pat": 2, "pde": 2}

            def make_at(h):
                return attn_pool.tile([128, S], B16, tag="at", name=f"at_{h}")

            qhs = {}
            ats_t = {}

            def q_thunks(wt, qh, h, sp):
                # One 1024-wide column pair (2 psums) of head h's Q
                # projection, as per-d thunks spliced between attention sk
                # tiles; lhsT reused for 2 consecutive matmuls.
                psq = [
                    ps_pool.tile([128, 512], F32, tag="p1", bufs=2,
                                 name=f"pq_{h}_{sp}_{s2}")
                    for s2 in range(2)
                ]

                def mk_d(d):
                    def thunk():
                        lhsT = wt[:, d * 128 : (d + 1) * 128]
                        for s2 in range(2):
                            s4 = 2 * sp + s2
                            nc.tensor.matmul(
                                psq[s2], lhsT=lhsT,
                                rhs=xts[d][:, s4 * 512 : (s4 + 1) * 512],
                                start=(d == 0), stop=(d == NT_D - 1),
                            )
                    return thunk

                def mk_epi(s2):
                    def thunk():
                        rope_epilogue(psq[s2], qh, 2 * sp + s2, "q", h, r_pool)
                    return thunk

                return [mk_d(d) for d in range(NT_D)] + [mk_epi(0), mk_epi(1)]

            for h in range(HPC):
                # Splice head h's Q-projection d-steps between head h-1's
                # attention sk tiles: the engine queues then carry
                # exp-independent matmuls through the softmax stretches.
                wt = load_w(wq, h, "q", w_pool)
                qh = qt_pool.tile([128, S], B16, tag="qt", name=f"qh_{h}")
                qhs[h] = qh
                fill0 = q_thunks(wt, qh, h, 0)
                fill1 = q_thunks(wt, qh, h, 1)
                if h == 0:
                    for t in fill0 + fill1:
                        t()
                else:
                    # s4 order puts mask-free full tiles first.
                    ats_t[h - 1] = make_at(h - 1)
                    attention(h - 1, qhs[h - 1], ats_t[h - 1], pt_pool,
                              rc_pool, ps_pool, P2B, s4_range=(1, 0),
                              fillers=fill0)
                    attention(h - 1, qhs.pop(h - 1), ats_t[h - 1], pt_pool,
                              rc_pool, ps_pool, P2B, s4_range=(3, 2),
                              fillers=fill1)
                    ats.append(ats_t[h - 1])

        # ------- Phase 3: attention(7) interleaved with Wo eo-loop -------
        # attention(7) has no Q-projection to hide under, so Wo matmuls fill
        # its exp-latency bubbles. The PSUM pool is shared (psc2/pat1/pde1 +
        # pwo4 = 8 banks). The first 3 eo-tiles accumulate only heads 0..6
        # and stage to SBUF so their banks recycle without waiting on
        # attention(7); their hv=7 term is added in a later fixup pass.
        ps3 = ctx.enter_context(tc.tile_pool(name="ps3", bufs=1, space="PSUM"))
        # 6 live Wo weight tiles at peak: 4 deferred-held + current + prefetch.
        wo_pool = ctx.enter_context(tc.tile_pool(name="wop", bufs=6, side="right"))
        P3B = {"psc": 2, "pat": 1, "pde": 1}
        NDEFER = 4

        def load_wo(eo):
            wot = wo_pool.tile([128, HPC * 128], B16, tag="wo",
                               name=f"wo_{eo}")
            nc.sync.dma_start(out=wot, in_=wo[eo])
            return wot

        at7 = attn_pool.tile([128, S], B16, tag="at", name=f"at_{HPC - 1}")
        qh7 = qhs.pop(HPC - 1)
        ats.append(at7)

        with tc.tile_pool(name="outp", bufs=6) as out_pool:
            wots = {0: load_wo(0)}

            def wo_psums(eo):
                return [
                    ps3.tile([128, 512], F32, tag="pwo", bufs=4,
                             name=f"pwo_{eo}_{s4}")
                    for s4 in range(NT_S4)
                ]

            def wo_accum(eo, psums, hv_range, start_hv, stop_hv):
                wot = wots[eo]
                for hv in hv_range:
                    lhsT = wot[:, hv * 128 : (hv + 1) * 128]
                    for s4 in range(NT_S4):
                        nc.tensor.matmul(
                            psums[s4], lhsT=lhsT,
                            rhs=ats[hv][:, s4 * 512 : (s4 + 1) * 512],
                            start=(hv == start_hv), stop=(hv == stop_hv),
                        )

            def wo_out(eo, s4, src_ap, add_ap=None):
                ot = out_pool.tile([128, 512], F32, tag="ot",
                                   name=f"ot_{eo}_{s4}")
                # DVE drain: keeps ACT free for attention(7)'s exp chain.
                if add_ap is None:
                    nc.vector.tensor_scalar_mul(ot, src_ap, 1.0)
                else:
                    nc.vector.tensor_add(out=ot, in0=src_ap, in1=add_ap)
                nc.sync.dma_start(
                    out=outt[eo * 128 : (eo + 1) * 128,
                             s4 * 512 : (s4 + 1) * 512],
                    in_=ot,
                )

            # Fine-grained interleave: the first NDEFER eo-tiles' head-0..6
            # accumulation is emitted as per-hv thunks spliced between
            # attention(7)'s sk tiles, so the engine queues carry independent
            # Wo matmuls through the exp-gated stretches.
            obufs = {}
            fillers = []
            psums_d = {}
            for eo in range(NDEFER):
                wots[eo + 1] = load_wo(eo + 1)
                psums_d[eo] = wo_psums(eo)

                def mk_hv(eo, hv):
                    def thunk():
                        wo_accum(eo, psums_d[eo], [hv], 0, HPC - 2)
                    return thunk

                def mk_drain(eo):
                    def thunk():
                        for s4b in range(NT_S4):
                            ob = out_pool.tile(
                                [128, 512], F32, tag=f"ob{eo}_{s4b}",
                                bufs=1, name=f"ob_{eo}_{s4b}")
                            nc.vector.tensor_scalar_mul(
                                ob, psums_d[eo][s4b], 1.0)
                            obufs[(eo, s4b)] = ob
                    return thunk

                for hv in range(HPC - 1):
                    fillers.append(mk_hv(eo, hv))
                fillers.append(mk_drain(eo))
            fillers.extend([lambda: None] * (40 - len(fillers)))

            attention(HPC - 1, qh7, at7, pt_pool, rc_pool,
                      ps3, P3B, s4_range=(3, 2, 1, 0), fillers=fillers)

            # eo = NDEFER: full accumulation (at7 is complete by now).
            for eo in range(NDEFER, NT_D):
                if eo + 1 < NT_D:
                    wots[eo + 1] = load_wo(eo + 1)
                psums = wo_psums(eo)
                if eo == NT_D - 1:
                    # Last tile: s4-outer so each psum drains + DMAs while the
                    # next s4 still accumulates (shortens the serial tail).
                    wot = wots[eo]
                    for s4 in range(NT_S4):
                        for hv in range(HPC):
                            nc.tensor.matmul(
                                psums[s4],
                                lhsT=wot[:, hv * 128 : (hv + 1) * 128],
                                rhs=ats[hv][:, s4 * 512 : (s4 + 1) * 512],
                                start=(hv == 0), stop=(hv == HPC - 1),
                            )
                        wo_out(eo, s4, psums[s4])
                    continue
                wo_accum(eo, psums, range(HPC), 0, HPC - 1)
                for s4 in range(NT_S4):
                    wo_out(eo, s4, psums[s4])
                if eo == NDEFER:
                    # Fixups: add the hv=7 term for the deferred eo-tiles and
                    # release their weight slots.
                    for feo in range(NDEFER):
                        psums_f = wo_psums(feo)
                        wot = wots[feo]
                        lhsT = wot[:, (HPC - 1) * 128 : HPC * 128]
                        for s4 in range(NT_S4):
                            nc.tensor.matmul(
                                psums_f[s4], lhsT=lhsT,
                                rhs=at7[:, s4 * 512 : (s4 + 1) * 512],
                                start=True, stop=True,
                            )
                        for s4 in range(NT_S4):
                            wo_out(feo, s4, psums_f[s4],
                                   add_ap=obufs[(feo, s4)])

    nc.compile()
    _PROGRAM_CACHE["nc"] = nc
    return nc


def _host_prep(x, Wq, Wk, Wv, Wo):
    """Shard + lay out inputs for the 8 cores. Returns list of in_maps."""
    # Within-head permutation: [even dk indices, odd dk indices] so the RoPE
    # pair (2i, 2i+1) becomes (row i, row 64+i) of each head's 128-row block.
    perm1 = np.concatenate([np.arange(0, DK, 2), np.arange(1, DK, 2)])
    perm = np.concatenate([h * DK + perm1 for h in range(H)])

    scale = 1.0 / math.sqrt(DK)
    WqP = (Wq * scale)[perm]          # fold 1/sqrt(dk) into Q
    WkP = Wk[perm]

    # RoPE tables in the permuted feature-major layout [128, S].
    inv_freq = 1.0 / (ROPE_THETA ** (np.arange(0, DK, 2, dtype=np.float64) / DK))
    ang = inv_freq[:, None] * np.arange(S, dtype=np.float64)[None, :]  # [64, S]
    cosP = np.vstack([np.cos(ang), np.cos(ang)]).astype(BF16)
    sinP = np.vstack([-np.sin(ang), np.sin(ang)]).astype(BF16)

    # Causal 0/1 mask: valid iff i <= j (only the r=0 pattern is needed).
    i_idx = np.arange(128)[:, None]
    j_idx = np.arange(512)[None, :]
    masks = np.ascontiguousarray((i_idx <= j_idx).astype(BF16))  # [128, 512]

    ones = np.ones((128, 128), dtype=BF16)

    def lhsT_blocks(Wt, n_out_tiles):
        # Wt: [contraction, width] (feature-major).
        # -> [n_out_tiles, 128, (contraction//128)*128] partition-major lhsT
        # tiles: [t, p, d*128 + f] = Wt[d*128 + p, t*w + f], so the device
        # DMA per tile is one contiguous [128, kt*128] block.
        kt = Wt.shape[0] // 128
        width = Wt.shape[1]
        blk = Wt.reshape(kt, 128, n_out_tiles, width // n_out_tiles)
        return np.ascontiguousarray(
            blk.transpose(2, 1, 0, 3).reshape(n_out_tiles, 128, kt * 128)
        ).astype(BF16)

    per_group = []
    for g in range(2):
        rows = slice(g * E, (g + 1) * E)
        wq_b = lhsT_blocks(WqP[rows].T, HPC)
        wk_b = lhsT_blocks(WkP[rows].T, HPC)
        wv_b = np.ascontiguousarray(
            Wv[rows].T.reshape(NT_D, 128, E)
        ).astype(BF16)
        # WoT [E, D]: lhsT blocks are [dv, e_out] tiles.
        wo_b = lhsT_blocks(np.ascontiguousarray(Wo[:, rows].T), NT_D)
        per_group.append((wq_b, wk_b, wv_b, wo_b))

    xts = []
    for b in range(B):
        xts.append(
            np.ascontiguousarray(x[b].T).astype(BF16).reshape(NT_D, 128, S)
        )

    in_maps = []
    for c in range(8):
        b, g = c // 2, c % 2
        wq_b, wk_b, wv_b, wo_b = per_group[g]
        in_maps.append(
            {
                "xt": xts[b],
                "wq": wq_b,
                "wk": wk_b,
                "wv": wv_b,
                "wo": wo_b,
                "cos": cosP,
                "sin": sinP,
                "msk": masks,
                "ones": ones,
            }
        )
    return in_maps


def kernel(x, Wq, Wk, Wv, Wo):
    global LAST_RESULT
    x = np.asarray(x, dtype=np.float32)
    Wq = np.asarray(Wq, dtype=np.float32)
    Wk = np.asarray(Wk, dtype=np.float32)
    Wv = np.asarray(Wv, dtype=np.float32)
    Wo = np.asarray(Wo, dtype=np.float32)

    if TRACE:
        _install_ntff_hook()

    from concourse.bass_utils import run_bass_kernel_spmd

    nc = _build_program()
    in_maps = _host_prep(x, Wq, Wk, Wv, Wo)
    res = run_bass_kernel_spmd(nc, in_maps, list(range(8)), trace=TRACE)
    LAST_RESULT = res

    out = np.empty((B, S, D), dtype=np.float32)
    for b in range(B):
        part = res.results[2 * b]["outt"] + res.results[2 * b + 1]["outt"]
        out[b] = part.T
    return out
